# revision 6
# baseline (speedup 1.0000x reference)
"""Trainium2 Bass kernel for the 2-layer GraphSAGE bus-stop predictor.

Self-contained: kernel(**inputs) -> np.ndarray [N, 1].

Strategy (8 NeuronCores, SPMD):
- Shard nodes by dst across 8 cores (125k nodes each).
- Aggregation (segment-mean) per layer via streaming dma_gather (256B rows,
  int16 window-local indices) -> SBUF batch -> dma_scatter_add (CCE add) into
  a per-core HBM accumulator. Calls are packed into (src-window x dst-quarter)
  cells, <=1024 descriptors per call (SWDGE ring limit), duplicate dst within
  a call deferred to the next call to avoid CCE RMW races.
- Dense phase per 512-node chunk: per-node 1/deg scale (DVE), PE transposes to
  feature-major, K=64 matmuls with BN-folded weights, ACT relu/sigmoid.
- Layer-1 output h1 is AllGathered (fp32) to form the full gather table for
  layer 2. Layer 3 (1-dim head) is fused into the layer-2 chunk loop.
"""

import time

import numpy as np

import concourse.bacc as bacc
import concourse.mybir as mybir
import concourse.tile as tile
from concourse.bass_utils import run_bass_kernel_spmd

f32 = mybir.dt.float32
i16 = mybir.dt.int16

N_CORES = 8
LAST_EXEC_NS = None
WIN = 32768          # gather window (int16 index reach)
CALL = 1024          # descriptors per SWDGE call (ring limit)
P = 128
CHUNK = 512          # dense-phase nodes per chunk

AF = mybir.ActivationFunctionType
OP = mybir.AluOpType


# ---------------------------------------------------------------- host prep

def _wrap_idx(vals):
    """int16 vals [CALL] -> dma_gather/scatter idx tile [128, CALL//16]."""
    m = np.asarray(vals, dtype=np.int16)
    assert m.shape == (CALL,)
    out = np.zeros((P, CALL // 16), dtype=np.int16)
    ii = np.arange(CALL)
    for g in range(8):
        out[(ii % 16) + 16 * g, ii // 16] = m
    return out


def _pack_cells(src, slotloc, n_table_rows, qsize):
    """Group edges into (window, quarter) cells; greedy-pack calls of exactly
    CALL entries with within-call unique dst; pad with (gather row 0 ->
    scatter dump slot qsize-1). slotloc is already in slot space."""
    dump = qsize - 1
    w = src // WIN
    q = slotloc // qsize
    dstloc = slotloc
    order = np.lexsort((dstloc, q, w))
    src, dstloc, w, q = src[order], dstloc[order], w[order], q[order]
    cells = {}
    nw = (n_table_rows + WIN - 1) // WIN
    bounds = np.searchsorted(w * 4 + q, np.arange(nw * 4 + 1))
    for cell in range(nw * 4):
        lo, hi = bounds[cell], bounds[cell + 1]
        if lo == hi:
            continue
        cw, cq = cell // 4, cell % 4
        s_loc = (src[lo:hi] - cw * WIN).astype(np.int16)
        d_loc = (dstloc[lo:hi] - cq * qsize).astype(np.int16)
        calls = []
        pend = list(zip(s_loc.tolist(), d_loc.tolist()))
        while pend:
            cs, cd = [], []
            seen = set()
            nxt = []
            for ss, dd in pend:
                if len(cs) < CALL and dd not in seen:
                    cs.append(ss)
                    cd.append(dd)
                    seen.add(dd)
                else:
                    nxt.append((ss, dd))
            while len(cs) < CALL:
                cs.append(0)
                cd.append(dump)
            calls.append((np.array(cs, np.int16), np.array(cd, np.int16)))
            pend = nxt
        cells[(cw, cq)] = calls
    return cells


def _schedule(all_cells, n_table_rows, qsize):
    """Uniform max-based schedule across cores."""
    nw = (n_table_rows + WIN - 1) // WIN
    meta = []
    per_core = [[] for _ in range(N_CORES)]
    for cell in sorted({k for c in all_cells for k in c}):
        n_calls = max(len(c.get(cell, [])) for c in all_cells)
        cw, cq = cell
        for k in range(n_calls):
            meta.append((cw * WIN, cq * qsize))
            for c in range(N_CORES):
                calls = all_cells[c].get(cell, [])
                if k < len(calls):
                    gs, ds = calls[k]
                else:
                    gs = np.zeros(CALL, np.int16)
                    ds = np.full(CALL, qsize - 1, np.int16)
                per_core[c].append(
                    np.stack([_wrap_idx(gs), _wrap_idx(ds)], axis=1))
    return meta, per_core


# ---------------------------------------------------------------- bass build

def _build(n_nodes, s, s_pad, qsize, xrows, meta1, meta2, n_chunks):
    nc = bacc.Bacc("TRN2", target_bir_lowering=False, debug=False,
                   num_devices=N_CORES)
    h_rows = N_CORES * s_pad

    x_pad = nc.dram_tensor("x_pad", [xrows, 64], f32, kind="ExternalInput")
    x_own = nc.dram_tensor("x_own", [s_pad, 64], f32, kind="ExternalInput")
    inv_in = nc.dram_tensor("inv_in", [n_chunks, P, 4], f32,
                            kind="ExternalInput")
    idx1 = nc.dram_tensor("idx1", [max(len(meta1), 1), P, 2, CALL // 16], i16,
                          kind="ExternalInput")
    idx2 = nc.dram_tensor("idx2", [max(len(meta2), 1), P, 2, CALL // 16], i16,
                          kind="ExternalInput")
    w_in = nc.dram_tensor("w_in", [64, 260], f32, kind="ExternalInput")
    out = nc.dram_tensor("out", [s_pad, 1], f32, kind="ExternalOutput")

    with tile.TileContext(nc) as tc:
        with tc.tile_pool(name="sb", bufs=1) as sb, \
             tc.tile_pool(name="ps", bufs=1, space="PSUM") as ps, \
             tc.tile_pool(name="dram", bufs=1, space="DRAM") as dr:

            agg1 = dr.tile([s_pad, 64], f32, tag="agg1", name="agg1")
            agg2 = dr.tile([s_pad, 64], f32, tag="agg2", name="agg2")
            h1 = dr.tile([s_pad, 64], f32, tag="h1", name="h1")
            h1f = dr.tile([h_rows, 64], f32, tag="h1f", name="h1f")

            # constants
            from concourse.masks import make_identity
            ident = sb.tile([P, P], f32, tag="ident", name="ident")
            make_identity(nc, ident[:])
            wts = sb.tile([64, 260], f32, tag="wts", name="wts")
            nc.sync.dma_start(out=wts[:], in_=w_in[:])
            # wts cols: 0-63 w1l_t, 64-127 w1r_t, 128-191 w2l_t, 192-255
            # w2r_t, 256 wp, 257 c1, 258 c2, 259 bp (row 0)
            zbuf = sb.tile([P, 16, 64], f32, tag="zbuf", name="zbuf")
            nc.vector.memset(zbuf[:], 0.0)

            # zero both aggs
            zrows = P * 16
            for base in range(0, s_pad, zrows):
                n = min(zrows, s_pad - base)
                t = n // P
                nc.sync.dma_start(
                    out=agg1[base:base + n, :]
                    .rearrange("(t p) d -> p t d", p=P),
                    in_=zbuf[:, :t, :])
                nc.sync.dma_start(
                    out=agg2[base:base + n, :]
                    .rearrange("(t p) d -> p t d", p=P),
                    in_=zbuf[:, :t, :])

            def agg_phase(meta, idx_dram, table, agg, label):
                for k, (wbase, qbase) in enumerate(meta):
                    it = sb.tile([P, 2, CALL // 16], i16,
                                 tag=f"it{k % 6}", name=f"it_{label}_{k}")
                    nc.sync.dma_start(out=it[:], in_=idx_dram[k])
                    buf = sb.tile([P, CALL // P, 64], f32,
                                  tag=f"gb{k % 4}", name=f"gb_{label}_{k}")
                    wrows = min(WIN, table.shape[0] - wbase)
                    nc.gpsimd.dma_gather(
                        out_ap=buf[:],
                        in_ap=table[wbase:wbase + wrows, :],
                        idxs_ap=it[:, 0, :],
                        num_idxs=CALL, num_idxs_reg=CALL, elem_size=64)
                    qrows = min(qsize, agg.shape[0] - qbase)
                    nc.gpsimd.dma_scatter_add(
                        out_ap=agg[qbase:qbase + qrows, :],
                        in_ap=buf[:], idxs_ap=it[:, 1, :],
                        num_idxs=CALL, num_idxs_reg=CALL, elem_size=64)

            def dense_phase(agg, root, wl, wr, bias_col, h_out, final):
                """Per-chunk: mean-scale, transpose, matmul, act.
                h_out: DRAM tile for node-major result (None if final).
                final: if True apply L3 head + sigmoid into out."""
                for c in range(n_chunks):
                    base = c * CHUNK
                    at = sb.tile([P, 4, 64], f32, tag="at", name=f"at{final}_{c}")
                    nc.sync.dma_start(
                        out=at[:],
                        in_=agg[base:base + CHUNK, :]
                        .rearrange("(t p) d -> p t d", p=P))
                    iv = sb.tile([P, 4], f32, tag="iv", name=f"iv{final}_{c}")
                    nc.sync.dma_start(out=iv[:], in_=inv_in[c])
                    for t in range(4):
                        nc.vector.tensor_scalar_mul(
                            at[:, t, :], at[:, t, :], iv[:, t:t + 1])
                    rt = sb.tile([P, 4, 64], f32, tag="rt", name=f"rt{final}_{c}")
                    nc.sync.dma_start(
                        out=rt[:],
                        in_=root[base:base + CHUNK, :]
                        .rearrange("(t p) d -> p t d", p=P))
                    pT = ps.tile([64, CHUNK], f32, tag="pT", name=f"pT{final}_{c}")
                    pR = ps.tile([64, CHUNK], f32, tag="pR", name=f"pR{final}_{c}")
                    for t in range(4):
                        nc.tensor.transpose(
                            out=pT[:, t * P:(t + 1) * P], in_=at[:, t, :],
                            identity=ident[:])
                        nc.tensor.transpose(
                            out=pR[:, t * P:(t + 1) * P], in_=rt[:, t, :],
                            identity=ident[:])
                    aT = sb.tile([64, CHUNK], f32, tag="aT", name=f"aT{final}_{c}")
                    rT = sb.tile([64, CHUNK], f32, tag="rT", name=f"rT{final}_{c}")
                    nc.scalar.copy(out=aT[:], in_=pT[:])
                    nc.scalar.copy(out=rT[:], in_=pR[:])
                    pm = ps.tile([64, CHUNK], f32, tag="pm", name=f"pm{final}_{c}")
                    nc.tensor.matmul(pm[:], lhsT=wts[:, wl * 64:wl * 64 + 64],
                                     rhs=aT[:], start=True, stop=False)
                    nc.tensor.matmul(pm[:], lhsT=wts[:, wr * 64:wr * 64 + 64],
                                     rhs=rT[:], start=False, stop=True)
                    hT = sb.tile([64, CHUNK], f32, tag="hT", name=f"hT{final}_{c}")
                    nc.scalar.activation(
                        hT[:], pm[:], AF.Relu,
                        bias=wts[:, 257 + bias_col:258 + bias_col], scale=1.0)
                    if not final:
                        pb = ps.tile([P, 4, 64], f32, tag="pb", name=f"pb{c}")
                        for t in range(4):
                            nc.tensor.transpose(
                                out=pb[:, t, :], in_=hT[:, t * P:(t + 1) * P],
                                identity=ident[:64, :64])
                        hn = sb.tile([P, 4, 64], f32, tag="hn", name=f"hn{c}")
                        nc.vector.tensor_copy(out=hn[:], in_=pb[:])
                        nc.sync.dma_start(
                            out=h_out[base:base + CHUNK, :]
                            .rearrange("(t p) d -> p t d", p=P),
                            in_=hn[:])
                    else:
                        po = ps.tile([1, CHUNK], f32, tag="po", name=f"po{c}")
                        nc.tensor.matmul(po[:], lhsT=wts[:, 256:257], rhs=hT[:],
                                         start=True, stop=True)
                        ob = sb.tile([1, CHUNK], f32, tag="ob", name=f"ob{c}")
                        nc.scalar.activation(
                            ob[:], po[:], AF.Sigmoid,
                            bias=wts[0:1, 259:260], scale=1.0)
                        nc.sync.dma_start(
                            out=out[base:base + CHUNK, :]
                            .rearrange("(o c) u -> o (c u)", o=1),
                            in_=ob[:])

            agg_phase(meta1, idx1, x_pad, agg1, "l1")
            dense_phase(agg1, x_own, 0, 1, 0, h1, final=False)

            import os
            if os.environ.get("NO_CC"):
                nc.sync.dma_start(out=h1f[:s_pad, :], in_=h1[:])
            else:
                nc.gpsimd.collective_compute(
                    "AllGather", OP.bypass,
                    replica_groups=[list(range(N_CORES))],
                    ins=[h1.opt()], outs=[h1f.opt()])

            agg_phase(meta2, idx2, h1f, agg2, "l2")
            dense_phase(agg2, h1, 2, 3, 1, None, final=True)

    nc.compile()
    return nc


# ---------------------------------------------------------------- exec


def _run_spmd_timed(nc, in_maps, n_iters=1):
    """Mirror bass2jax.run_bass_via_pjrt but pre-upload inputs so the timed
    exec excludes H2D. Returns (results, best_exec_ns)."""
    import jax
    import numpy as _np
    from jax.sharding import Mesh, PartitionSpec, NamedSharding
    from jax.experimental.shard_map import shard_map
    from concourse import bass2jax as b2j
    import concourse.mybir as _mb

    b2j.install_neuronx_cc_hook()
    n_cores = len(in_maps)
    partition_name = (nc.partition_id_tensor.name
                      if nc.partition_id_tensor else None)
    in_names, out_names, out_avals, zero_outs = [], [], [], []
    for alloc in nc.m.functions[0].allocations:
        if not isinstance(alloc, _mb.MemoryLocationSet):
            continue
        name = alloc.memorylocations[0].name
        if alloc.kind == "ExternalInput":
            if name != partition_name:
                in_names.append(name)
        elif alloc.kind == "ExternalOutput":
            shape = tuple(alloc.tensor_shape)
            dtype = _mb.dt.np(alloc.dtype)
            out_names.append(name)
            out_avals.append(jax.core.ShapedArray(shape, dtype))
            zero_outs.append(_np.zeros(shape, dtype))
    n_params = len(in_names)
    all_in_names = list(in_names) + list(out_names)
    if partition_name is not None:
        all_in_names.append(partition_name)

    def _body(*args):
        operands = list(args)
        if partition_name is not None:
            operands.append(b2j.partition_id_tensor())
        return tuple(b2j._bass_exec_p.bind(
            *operands,
            out_avals=tuple(out_avals),
            in_names=tuple(all_in_names),
            out_names=tuple(out_names),
            lowering_input_output_aliases=(),
            sim_require_finite=True,
            sim_require_nnan=True,
            nc=nc,
        ))

    devices = jax.devices()[:n_cores]
    mesh = Mesh(_np.asarray(devices), ("core",))
    spec = PartitionSpec("core")
    sharding = NamedSharding(mesh, spec)
    donate = tuple(range(n_params, n_params + len(out_names)))
    sharded = jax.jit(
        shard_map(_body, mesh=mesh, in_specs=(spec,) * (n_params + len(zero_outs)),
                  out_specs=(spec,) * len(out_names), check_rep=False),
        donate_argnums=donate, keep_unused=True)

    concat_in = [
        _np.concatenate([_np.asarray(in_maps[c][name]) for c in range(n_cores)],
                        axis=0)
        for name in in_names
    ]
    dev_in = [jax.device_put(a, sharding) for a in concat_in]
    for a in dev_in:
        a.block_until_ready()

    def zeros_dev():
        zs = [jax.device_put(
            _np.zeros((n_cores * z.shape[0], *z.shape[1:]), z.dtype), sharding)
            for z in zero_outs]
        for z in zs:
            z.block_until_ready()
        return zs

    best = None
    outs = None
    for _ in range(max(1, n_iters)):
        zs = zeros_dev()
        t0 = time.time()
        res = sharded(*dev_in, *zs)
        for r in res:
            r.block_until_ready()
        dt = (time.time() - t0) * 1e9
        if best is None or dt < best:
            best = dt
        outs = res
    results = [
        {name: _np.asarray(outs[i]).reshape(n_cores, *out_avals[i].shape)[c]
         for i, name in enumerate(out_names)}
        for c in range(n_cores)
    ]
    return results, best


# ---------------------------------------------------------------- entry

def prepare(x, edge_index, W1l, b1, W1r, g1, be1, rm1, rv1,
            W2l, b2, W2r, g2, be2, rm2, rv2, Wp, bp):
    """Host prep + bass build. Returns (nc, in_maps, slot_of, s_pad, n)."""
    _t0 = time.time()
    x = np.asarray(x, np.float32)
    edge_index = np.asarray(edge_index)
    n = x.shape[0]
    src = edge_index[0].astype(np.int64)
    dst = edge_index[1].astype(np.int64)
    s = n // N_CORES
    qn = (s + 3) // 4                       # nodes per quarter
    qsize = ((qn + 1 + CHUNK - 1) // CHUNK) * CHUNK   # >=1 spare dump slot
    assert qsize <= 32767
    s_pad = 4 * qsize
    n_chunks = s_pad // CHUNK

    def to_slot(nloc):
        q = nloc // qn
        return q * qsize + (nloc - q * qn)
    eps = 1e-5

    # BN fold
    s1 = (np.asarray(g1) / np.sqrt(np.asarray(rv1) + eps)).astype(np.float32)
    s2 = (np.asarray(g2) / np.sqrt(np.asarray(rv2) + eps)).astype(np.float32)
    w1l = (s1[:, None] * np.asarray(W1l)).astype(np.float32)   # [64, 4]
    w1r = (s1[:, None] * np.asarray(W1r)).astype(np.float32)
    c1 = (np.asarray(be1) + (np.asarray(b1) - np.asarray(rm1)) * s1
          ).astype(np.float32)
    w2l = (s2[:, None] * np.asarray(W2l)).astype(np.float32)   # [64, 64]
    w2r = (s2[:, None] * np.asarray(W2r)).astype(np.float32)
    c2 = (np.asarray(be2) + (np.asarray(b2) - np.asarray(rm2)) * s2
          ).astype(np.float32)

    wts = np.zeros((64, 260), np.float32)
    wts[:4, 0:64] = w1l.T
    wts[:4, 64:128] = w1r.T
    wts[:, 128:192] = w2l.T
    wts[:, 192:256] = w2r.T
    wts[:, 256] = np.asarray(Wp, np.float32)[0]
    wts[:, 257] = c1
    wts[:, 258] = c2
    wts[0, 259] = np.float32(np.asarray(bp).ravel()[0])

    deg = np.bincount(dst, minlength=n).astype(np.float32)
    inv = 1.0 / np.maximum(deg, 1.0)

    x_pad = np.zeros((n, 64), np.float32)
    x_pad[:, :4] = x

    # per-core edge prep
    h_rows = N_CORES * s_pad
    slot_of = to_slot(np.arange(s))
    cells1, cells2 = [], []
    for d in range(N_CORES):
        m = (dst // s) == d
        sd = src[m]
        sl = to_slot(dst[m] - d * s)
        cells1.append(_pack_cells(sd, sl, n, qsize))
        # L2: table rows in the padded slot-space allgather layout
        sg = (sd // s) * s_pad + to_slot(sd % s)
        cells2.append(_pack_cells(sg, sl, h_rows, qsize))
    print(f"[kernel] edge prep done {time.time()-_t0:.0f}s", flush=True)
    meta1, idx1 = _schedule(cells1, n, qsize)
    meta2, idx2 = _schedule(cells2, h_rows, qsize)
    print(f"[kernel] schedule done: calls L1={len(meta1)} L2={len(meta2)} "
          f"{time.time()-_t0:.0f}s", flush=True)

    nc = _build(n, s, s_pad, qsize, n, meta1, meta2, n_chunks)
    print(f"[kernel] bass build+compile done {time.time()-_t0:.0f}s", flush=True)

    in_maps = []
    for d in range(N_CORES):
        x_own = np.zeros((s_pad, 64), np.float32)
        x_own[slot_of] = x_pad[d * s:(d + 1) * s]
        iv = np.zeros(s_pad, np.float32)
        iv[slot_of] = inv[d * s:(d + 1) * s]
        inv_t = iv.reshape(n_chunks, 4, P).transpose(0, 2, 1).copy()
        in_maps.append({
            "x_pad": x_pad,
            "x_own": x_own,
            "inv_in": inv_t,
            "idx1": np.stack(idx1[d]) if idx1[d] else
            np.zeros((1, P, 2, CALL // 16), np.int16),
            "idx2": np.stack(idx2[d]) if idx2[d] else
            np.zeros((1, P, 2, CALL // 16), np.int16),
            "w_in": wts,
        })

    print(f"[kernel] inputs packed {time.time()-_t0:.0f}s", flush=True)
    return nc, in_maps, slot_of, s_pad, n


def kernel(x, edge_index, W1l, b1, W1r, g1, be1, rm1, rv1,
           W2l, b2, W2r, g2, be2, rm2, rv2, Wp, bp, _sim=False):
    _t0 = time.time()
    nc, in_maps, slot_of, s_pad, n = prepare(
        x, edge_index, W1l, b1, W1r, g1, be1, rm1, rv1,
        W2l, b2, W2r, g2, be2, rm2, rv2, Wp, bp)
    if _sim:
        import concourse.bass_interp as bass_interp
        sim = bass_interp.MultiCoreSim(nc, N_CORES)
        for d in range(N_CORES):
            for k, v in in_maps[d].items():
                sim.cores[d].tensor(k)[:] = v.reshape(
                    sim.cores[d].tensor(k).shape)
        sim.simulate(check_with_hw=False)
        outs = [np.asarray(sim.cores[d].mem_tensor("out")).reshape(s_pad)[slot_of]
                for d in range(N_CORES)]
        return np.concatenate(outs).reshape(n, 1).astype(np.float32)
    global LAST_EXEC_NS
    import os
    n_iters = int(os.environ.get("BENCH_ITERS", "1"))
    results, best_ns = _run_spmd_timed(nc, in_maps, n_iters=n_iters)
    LAST_EXEC_NS = best_ns
    print(f"[kernel] exec done {time.time()-_t0:.0f}s", flush=True)
    outs = [results[d]["out"][slot_of, 0] for d in range(N_CORES)]
    return np.concatenate(outs).reshape(n, 1).astype(np.float32)



# revision 14
# speedup vs baseline: 1.0272x; 1.0272x over previous
"""Trainium2 Bass kernel for the 2-layer GraphSAGE bus-stop predictor.

Self-contained: kernel(**inputs) -> np.ndarray [N, 1].

Strategy (8 NeuronCores, SPMD):
- Shard nodes by dst across 8 cores (125k nodes each).
- Aggregation (segment-mean) per layer via streaming dma_gather (256B rows,
  int16 window-local indices) -> SBUF batch -> dma_scatter_add (CCE add) into
  a per-core HBM accumulator with TWO slots per node (occ 0/1), so duplicate
  dst within a call never race (different addresses; >=3rd occurrence spills
  to a later wave). Calls are packed per (src-window x dst-eighth) cell.
- Dense phase per 512-node chunk in bf16: 2-slot sum + 1/deg scale (DVE),
  PE transposes to feature-major, K=64 matmuls with BN-folded bf16 weights,
  ACT relu/sigmoid. Root term uses pre-transposed bf16 tables (x_own_T from
  host; h1T written by layer 1), eliminating root transposes.
- Layer-1 output h1 (fp32 node-major) is AllGathered to form the layer-2
  gather table. The 1-dim head is fused into the layer-2 chunk loop.
"""

import time

import numpy as np

import concourse.bacc as bacc
import concourse.mybir as mybir
import concourse.tile as tile
from concourse.bass_utils import run_bass_kernel_spmd

f32 = mybir.dt.float32
bf16 = mybir.dt.bfloat16
i16 = mybir.dt.int16

N_CORES = 8
LAST_EXEC_NS = None
WIN = 32768          # gather window (int16 index reach)
CALL = 1024          # idxs per SWDGE call (ring limit; 2048 desyncs)
SCRATCH = 16384      # dynamic_dma_scratch_size (SWDGE ring, default)
P = 128
CHUNK = 512          # dense-phase nodes per chunk
SEGS = 8             # dst segments per core (scatter windows)

AF = mybir.ActivationFunctionType
OP = mybir.AluOpType


# ---------------------------------------------------------------- host prep

def _wrap_idx(vals, call=None):
    """int16 vals [call] -> dma idx tile [128, call//16]."""
    m = np.asarray(vals, dtype=np.int16)
    call = call or m.shape[0]
    assert m.shape == (call,)
    out = np.zeros((P, call // 16), dtype=np.int16)
    ii = np.arange(call)
    for g in range(8):
        out[(ii % 16) + 16 * g, ii // 16] = m
    return out


def _pack_cells(src, dstloc, segn, segspan):
    """Group one core's edges into (window, segment) cells; pack calls of
    exactly CALL entries allowing <=2 occurrences per dst per call (2-slot
    accumulator); pad with (gather row 0 -> scatter dump slot segspan-1).

    Returns dict (w, seg) -> list of (gidx, sidx) int16 arrays."""
    dump = segspan - 1
    w = (src // WIN).astype(np.int64)
    seg = (dstloc // segn).astype(np.int64)
    order = np.lexsort((dstloc, seg, w))
    src, dstloc, w, seg = src[order], dstloc[order], w[order], seg[order]
    cells = {}
    key = w * SEGS + seg
    bounds = np.searchsorted(key, np.arange(key[-1] + 2 if len(key) else 1))
    uniq = np.unique(key)
    for cell in uniq:
        lo, hi = bounds[cell], bounds[cell + 1]
        cw, cs = int(cell) // SEGS, int(cell) % SEGS
        s_loc = (src[lo:hi] - cw * WIN).astype(np.int16)
        d = dstloc[lo:hi] - cs * segn
        # rank of each edge within its dst run (dst-sorted within cell)
        starts = np.flatnonzero(np.concatenate(([True], d[1:] != d[:-1])))
        run_id = np.cumsum(np.concatenate(
            ([False], d[1:] != d[:-1]))).astype(np.int64)
        rank = np.arange(len(d)) - starts[run_id]
        occ = (rank % 2).astype(np.int16)
        wave = (rank // 2).astype(np.int64)
        sidx_all = (2 * d + occ).astype(np.int16)
        calls = []
        for wv in range(int(wave.max()) + 1 if len(wave) else 0):
            m = wave == wv
            gs, ds = s_loc[m], sidx_all[m]
            for base in range(0, len(gs), CALL):
                calls.append((gs[base:base + CALL], ds[base:base + CALL]))
        cells[(cw, cs)] = calls
    return cells


def _schedule(all_cells, segspan):
    """Uniform max-based schedule across cores; per-call size = cross-core
    max rounded up to 128 (pads: gather row 0 -> scatter dump slot)."""
    dump = segspan - 1
    meta = []
    per_core = [[] for _ in range(N_CORES)]
    for cell in sorted({k for c in all_cells for k in c}):
        n_calls = max(len(c.get(cell, [])) for c in all_cells)
        cw, cs = cell
        for k in range(n_calls):
            lens = []
            for c in range(N_CORES):
                calls = all_cells[c].get(cell, [])
                lens.append(len(calls[k][0]) if k < len(calls) else 0)
            sz = max(128, -(-max(lens) // 128) * 128)
            meta.append((cw * WIN, cs * segspan, sz))
            for c in range(N_CORES):
                calls = all_cells[c].get(cell, [])
                if k < len(calls):
                    gs, ds = calls[k]
                else:
                    gs = np.empty(0, np.int16)
                    ds = np.empty(0, np.int16)
                pad = sz - len(gs)
                g_c = np.concatenate([gs, np.zeros(pad, np.int16)])
                d_c = np.concatenate([ds, np.full(pad, dump, np.int16)])
                tile_g = _wrap_idx(np.concatenate(
                    [g_c, np.zeros(CALL - sz, np.int16)]), CALL)
                tile_d = _wrap_idx(np.concatenate(
                    [d_c, np.full(CALL - sz, dump, np.int16)]), CALL)
                per_core[c].append(np.stack([tile_g, tile_d], axis=1))
    return meta, per_core


# ---------------------------------------------------------------- bass build

def _build(segn, segspan, s_hpad, xrows, h_rows, meta1, meta2, n_chunks):
    nc = bacc.Bacc("TRN2", target_bir_lowering=False, debug=False,
                   num_devices=N_CORES, dynamic_dma_scratch_size=SCRATCH)
    agg_rows = SEGS * segspan
    cpseg = segn // CHUNK    # dense chunks per segment

    x_pad = nc.dram_tensor("x_pad", [xrows, 64], f32, kind="ExternalInput")
    xT = nc.dram_tensor("xT", [64, s_hpad], bf16, kind="ExternalInput")
    inv_in = nc.dram_tensor("inv_in", [n_chunks, P, 4], f32,
                            kind="ExternalInput")
    idx1 = nc.dram_tensor("idx1", [max(len(meta1), 1), P, 2, CALL // 16], i16,
                          kind="ExternalInput")
    idx2 = nc.dram_tensor("idx2", [max(len(meta2), 1), P, 2, CALL // 16], i16,
                          kind="ExternalInput")
    wb_in = nc.dram_tensor("wb_in", [64, 260], bf16, kind="ExternalInput")
    wf_in = nc.dram_tensor("wf_in", [64, 4], f32, kind="ExternalInput")
    out = nc.dram_tensor("out", [s_hpad, 1], f32, kind="ExternalOutput")

    with tile.TileContext(nc) as tc:
        with tc.tile_pool(name="sb", bufs=1) as sb, \
             tc.tile_pool(name="ps", bufs=1, space="PSUM") as ps, \
             tc.tile_pool(name="dram", bufs=1, space="DRAM") as dr:

            agg1 = dr.tile([agg_rows, 64], f32, tag="agg1", name="agg1")
            agg2 = dr.tile([agg_rows, 64], f32, tag="agg2", name="agg2")
            h1 = dr.tile([s_hpad, 64], f32, tag="h1", name="h1")
            h1T = dr.tile([64, s_hpad], bf16, tag="h1T", name="h1T")
            h1f = dr.tile([h_rows, 64], f32, tag="h1f", name="h1f")

            # constants
            from concourse.masks import make_identity
            identb = sb.tile([P, P], bf16, tag="identb", name="identb")
            make_identity(nc, identb[:])
            wb = sb.tile([64, 260], bf16, tag="wb", name="wb")
            nc.sync.dma_start(out=wb[:], in_=wb_in[:])
            # wb cols: 0-63 w1l_t, 64-127 w1r_t, 128-191 w2l_t,
            # 192-255 w2r_t, 256 wp
            wf = sb.tile([64, 4], f32, tag="wf", name="wf")
            nc.sync.dma_start(out=wf[:], in_=wf_in[:])
            # wf cols: 0 c1, 1 c2, 2 bp(row0)
            zbuf = sb.tile([P, 16, 64], f32, tag="zbuf", name="zbuf")
            nc.vector.memset(zbuf[:], 0.0)

            def zero_agg(agg):
                zrows = P * 16
                for s in range(SEGS):
                    rows = 2 * segn
                    for base in range(0, rows, zrows):
                        nr = min(zrows, rows - base)
                        t = nr // P
                        nc.sync.dma_start(
                            out=agg[s * segspan + base:
                                    s * segspan + base + nr, :]
                            .rearrange("(t p) d -> p t d", p=P),
                            in_=zbuf[:, :t, :])

            zero_agg(agg1)
            zero_agg(agg2)

            def agg_phase(meta, idx_dram, table, agg, label):
                for k, (wbase, sbase, sz) in enumerate(meta):
                    it = sb.tile([P, 2, CALL // 16], i16,
                                 tag=f"it{k % 6}", name=f"it_{label}_{k}")
                    nc.sync.dma_start(out=it[:, :, :sz // 16],
                                      in_=idx_dram[k, :, :, :sz // 16])
                    buf = sb.tile([P, CALL // P, 64], f32,
                                  tag=f"gb{k % 4}", name=f"gb_{label}_{k}")
                    wrows = min(WIN, table.shape[0] - wbase)
                    nc.gpsimd.dma_gather(
                        out_ap=buf[:, :sz // P, :],
                        in_ap=table[wbase:wbase + wrows, :],
                        idxs_ap=it[:, 0, :sz // 16],
                        num_idxs=sz, num_idxs_reg=sz, elem_size=64)
                    nc.gpsimd.dma_scatter_add(
                        out_ap=agg[sbase:sbase + segspan, :],
                        in_ap=buf[:, :sz // P, :], idxs_ap=it[:, 1, :sz // 16],
                        num_idxs=sz, num_idxs_reg=sz, elem_size=64)

            def dense_phase(agg, rootT, wl, wr, bias_col, final):
                """Per-chunk: 2-slot sum, 1/deg scale, transpose, matmul, act.
                rootT: [64, s_hpad] bf16 table for the root term.
                final: if True apply L3 head + sigmoid into out, else write
                h1 (fp32 node-major) + h1T (bf16 feature-major)."""
                for c in range(n_chunks):
                    seg, cs = c // cpseg, c % cpseg
                    abase = seg * segspan + cs * 2 * CHUNK
                    base = c * CHUNK
                    a2 = sb.tile([P, 4, 2, 64], f32, tag="a2",
                                 name=f"a2{final}_{c}")
                    nc.sync.dma_start(
                        out=a2[:],
                        in_=agg[abase:abase + 2 * CHUNK, :]
                        .rearrange("(t p two) d -> p t two d", p=P, two=2))
                    at = sb.tile([P, 4, 64], f32, tag="at",
                                 name=f"at{final}_{c}")
                    nc.vector.tensor_tensor(
                        out=at[:], in0=a2[:, :, 0, :], in1=a2[:, :, 1, :],
                        op=OP.add)
                    iv = sb.tile([P, 4], f32, tag="iv", name=f"iv{final}_{c}")
                    nc.sync.dma_start(out=iv[:], in_=inv_in[c])
                    ab = sb.tile([P, 4, 64], bf16, tag="ab",
                                 name=f"ab{final}_{c}")
                    for t in range(4):
                        nc.vector.tensor_scalar_mul(
                            ab[:, t, :], at[:, t, :], iv[:, t:t + 1])
                    pT = ps.tile([64, CHUNK], bf16, tag="pT",
                                 name=f"pT{final}_{c}")
                    for t in range(4):
                        nc.tensor.transpose(
                            out=pT[:, t * P:(t + 1) * P], in_=ab[:, t, :],
                            identity=identb[:])
                    aT = sb.tile([64, CHUNK], bf16, tag="aT",
                                 name=f"aT{final}_{c}")
                    nc.scalar.copy(out=aT[:], in_=pT[:])
                    rT = sb.tile([64, CHUNK], bf16, tag="rT",
                                 name=f"rT{final}_{c}")
                    nc.sync.dma_start(out=rT[:],
                                      in_=rootT[:, base:base + CHUNK])
                    pm = ps.tile([64, CHUNK], f32, tag="pm",
                                 name=f"pm{final}_{c}")
                    nc.tensor.matmul(pm[:], lhsT=wb[:, wl * 64:wl * 64 + 64],
                                     rhs=aT[:], start=True, stop=False)
                    nc.tensor.matmul(pm[:], lhsT=wb[:, wr * 64:wr * 64 + 64],
                                     rhs=rT[:], start=False, stop=True)
                    hT = sb.tile([64, CHUNK], bf16, tag="hT",
                                 name=f"hT{final}_{c}")
                    nc.scalar.activation(
                        hT[:], pm[:], AF.Relu,
                        bias=wf[:, bias_col:bias_col + 1], scale=1.0)
                    if not final:
                        nc.sync.dma_start(out=h1T[:, base:base + CHUNK],
                                          in_=hT[:])
                        pb = ps.tile([P, 4, 64], bf16, tag="pb", name=f"pb{c}")
                        for t in range(4):
                            nc.tensor.transpose(
                                out=pb[:, t, :], in_=hT[:, t * P:(t + 1) * P],
                                identity=identb[:64, :64])
                        hn = sb.tile([P, 4, 64], f32, tag="hn", name=f"hn{c}")
                        nc.vector.tensor_copy(out=hn[:], in_=pb[:])
                        nc.sync.dma_start(
                            out=h1[base:base + CHUNK, :]
                            .rearrange("(t p) d -> p t d", p=P),
                            in_=hn[:])
                    else:
                        po = ps.tile([1, CHUNK], f32, tag="po", name=f"po{c}")
                        nc.tensor.matmul(po[:], lhsT=wb[:, 256:257], rhs=hT[:],
                                         start=True, stop=True)
                        ob = sb.tile([1, CHUNK], f32, tag="ob", name=f"ob{c}")
                        nc.scalar.activation(
                            ob[:], po[:], AF.Sigmoid,
                            bias=wf[0:1, 2:3], scale=1.0)
                        nc.sync.dma_start(
                            out=out[base:base + CHUNK, :]
                            .rearrange("(o c) u -> o (c u)", o=1),
                            in_=ob[:])

            agg_phase(meta1, idx1, x_pad, agg1, "l1")
            dense_phase(agg1, xT, 0, 1, 0, final=False)

            import os
            if os.environ.get("NO_CC"):
                nc.sync.dma_start(out=h1f[:s_hpad, :], in_=h1[:])
            else:
                nc.gpsimd.collective_compute(
                    "AllGather", OP.bypass,
                    replica_groups=[list(range(N_CORES))],
                    ins=[h1.opt()], outs=[h1f.opt()])

            agg_phase(meta2, idx2, h1f, agg2, "l2")
            dense_phase(agg2, h1T, 2, 3, 1, final=True)

    nc.compile()
    return nc


# ---------------------------------------------------------------- exec


def _run_spmd_timed(nc, in_maps, n_iters=1):
    """Mirror bass2jax.run_bass_via_pjrt but pre-upload inputs so the timed
    exec excludes H2D. Returns (results, best_exec_ns)."""
    import jax
    import numpy as _np
    from jax.sharding import Mesh, PartitionSpec, NamedSharding
    from jax.experimental.shard_map import shard_map
    from concourse import bass2jax as b2j
    import concourse.mybir as _mb

    b2j.install_neuronx_cc_hook()
    n_cores = len(in_maps)
    partition_name = (nc.partition_id_tensor.name
                      if nc.partition_id_tensor else None)
    in_names, out_names, out_avals, zero_outs = [], [], [], []
    for alloc in nc.m.functions[0].allocations:
        if not isinstance(alloc, _mb.MemoryLocationSet):
            continue
        name = alloc.memorylocations[0].name
        if alloc.kind == "ExternalInput":
            if name != partition_name:
                in_names.append(name)
        elif alloc.kind == "ExternalOutput":
            shape = tuple(alloc.tensor_shape)
            dtype = _mb.dt.np(alloc.dtype)
            out_names.append(name)
            out_avals.append(jax.core.ShapedArray(shape, dtype))
            zero_outs.append(_np.zeros(shape, dtype))
    n_params = len(in_names)
    all_in_names = list(in_names) + list(out_names)
    if partition_name is not None:
        all_in_names.append(partition_name)

    def _body(*args):
        operands = list(args)
        if partition_name is not None:
            operands.append(b2j.partition_id_tensor())
        return tuple(b2j._bass_exec_p.bind(
            *operands,
            out_avals=tuple(out_avals),
            in_names=tuple(all_in_names),
            out_names=tuple(out_names),
            lowering_input_output_aliases=(),
            sim_require_finite=True,
            sim_require_nnan=True,
            nc=nc,
        ))

    devices = jax.devices()[:n_cores]
    mesh = Mesh(_np.asarray(devices), ("core",))
    spec = PartitionSpec("core")
    sharding = NamedSharding(mesh, spec)
    donate = tuple(range(n_params, n_params + len(out_names)))
    sharded = jax.jit(
        shard_map(_body, mesh=mesh,
                  in_specs=(spec,) * (n_params + len(zero_outs)),
                  out_specs=(spec,) * len(out_names), check_rep=False),
        donate_argnums=donate, keep_unused=True)

    concat_in = [
        _np.concatenate([_np.asarray(in_maps[c][name])
                         for c in range(n_cores)], axis=0)
        for name in in_names
    ]
    dev_in = [jax.device_put(a, sharding) for a in concat_in]
    for a in dev_in:
        a.block_until_ready()

    def zeros_dev():
        zs = [jax.device_put(
            _np.zeros((n_cores * z.shape[0], *z.shape[1:]), z.dtype),
            sharding) for z in zero_outs]
        for z in zs:
            z.block_until_ready()
        return zs

    best = None
    outs = None
    for _ in range(max(1, n_iters)):
        zs = zeros_dev()
        t0 = time.time()
        res = sharded(*dev_in, *zs)
        for r in res:
            r.block_until_ready()
        dt = (time.time() - t0) * 1e9
        if best is None or dt < best:
            best = dt
        outs = res
    results = [
        {name: _np.asarray(outs[i]).reshape(n_cores, *out_avals[i].shape)[c]
         for i, name in enumerate(out_names)}
        for c in range(n_cores)
    ]
    return results, best


# ---------------------------------------------------------------- entry


def prepare(x, edge_index, W1l, b1, W1r, g1, be1, rm1, rv1,
            W2l, b2, W2r, g2, be2, rm2, rv2, Wp, bp):
    """Host prep + bass build. Returns (nc, in_maps, s, s_hpad, n)."""
    _t0 = time.time()
    x = np.asarray(x, np.float32)
    edge_index = np.asarray(edge_index)
    n = x.shape[0]
    src = edge_index[0].astype(np.int64)
    dst = edge_index[1].astype(np.int64)
    s = n // N_CORES                         # 125000
    segn = -(-s // SEGS)                     # nodes per segment
    segn = ((segn + CHUNK - 1) // CHUNK) * CHUNK      # align to CHUNK
    s_hpad = SEGS * segn                     # padded compact rows per core
    segspan = 2 * segn + CHUNK               # 2 slots + spare dump space
    assert segspan <= 32767
    n_chunks = s_hpad // CHUNK
    h_rows = N_CORES * s_hpad
    eps = 1e-5

    # BN fold
    s1 = (np.asarray(g1) / np.sqrt(np.asarray(rv1) + eps)).astype(np.float32)
    s2 = (np.asarray(g2) / np.sqrt(np.asarray(rv2) + eps)).astype(np.float32)
    w1l = (s1[:, None] * np.asarray(W1l)).astype(np.float32)   # [64, 4]
    w1r = (s1[:, None] * np.asarray(W1r)).astype(np.float32)
    c1 = (np.asarray(be1) + (np.asarray(b1) - np.asarray(rm1)) * s1
          ).astype(np.float32)
    w2l = (s2[:, None] * np.asarray(W2l)).astype(np.float32)   # [64, 64]
    w2r = (s2[:, None] * np.asarray(W2r)).astype(np.float32)
    c2 = (np.asarray(be2) + (np.asarray(b2) - np.asarray(rm2)) * s2
          ).astype(np.float32)

    wtsb = np.zeros((64, 260), np.float32)
    wtsb[:4, 0:64] = w1l.T
    wtsb[:4, 64:128] = w1r.T
    wtsb[:, 128:192] = w2l.T
    wtsb[:, 192:256] = w2r.T
    wtsb[:, 256] = np.asarray(Wp, np.float32)[0]
    import ml_dtypes
    wtsb = wtsb.astype(ml_dtypes.bfloat16)
    wtsf = np.zeros((64, 4), np.float32)
    wtsf[:, 0] = c1
    wtsf[:, 1] = c2
    wtsf[0, 2] = np.float32(np.asarray(bp).ravel()[0])

    deg = np.bincount(dst, minlength=n).astype(np.float32)
    inv = 1.0 / np.maximum(deg, 1.0)

    x_pad = np.zeros((n, 64), np.float32)
    x_pad[:, :4] = x

    # per-core edge prep
    cells1, cells2 = [], []
    for d in range(N_CORES):
        m = (dst // s) == d
        sd = src[m]
        dl = dst[m] - d * s
        cells1.append(_pack_cells(sd, dl, segn, segspan))
        # L2: table rows in the compact allgather layout
        sg = (sd // s) * s_hpad + (sd % s)
        cells2.append(_pack_cells(sg, dl, segn, segspan))
    print(f"[kernel] edge prep done {time.time()-_t0:.0f}s", flush=True)
    meta1, idx1 = _schedule(cells1, segspan)
    meta2, idx2 = _schedule(cells2, segspan)
    print(f"[kernel] schedule done: calls L1={len(meta1)} L2={len(meta2)} "
          f"{time.time()-_t0:.0f}s", flush=True)

    nc = _build(segn, segspan, s_hpad, n, h_rows, meta1, meta2, n_chunks)
    print(f"[kernel] bass build+compile done {time.time()-_t0:.0f}s",
          flush=True)

    in_maps = []
    for d in range(N_CORES):
        xT = np.zeros((64, s_hpad), np.float32)
        xT[:4, :s] = x[d * s:(d + 1) * s].T
        iv = np.zeros(s_hpad, np.float32)
        iv[:s] = inv[d * s:(d + 1) * s]
        inv_t = iv.reshape(n_chunks, 4, P).transpose(0, 2, 1).copy()
        in_maps.append({
            "x_pad": x_pad,
            "xT": xT.astype(ml_dtypes.bfloat16),
            "inv_in": inv_t,
            "idx1": np.stack(idx1[d]) if idx1[d] else
            np.zeros((1, P, 2, CALL // 16), np.int16),
            "idx2": np.stack(idx2[d]) if idx2[d] else
            np.zeros((1, P, 2, CALL // 16), np.int16),
            "wb_in": wtsb,
            "wf_in": wtsf,
        })
    print(f"[kernel] inputs packed {time.time()-_t0:.0f}s", flush=True)
    return nc, in_maps, s, s_hpad, n


def kernel(x, edge_index, W1l, b1, W1r, g1, be1, rm1, rv1,
           W2l, b2, W2r, g2, be2, rm2, rv2, Wp, bp, _sim=False):
    _t0 = time.time()
    nc, in_maps, s, s_hpad, n = prepare(
        x, edge_index, W1l, b1, W1r, g1, be1, rm1, rv1,
        W2l, b2, W2r, g2, be2, rm2, rv2, Wp, bp)
    if _sim:
        import concourse.bass_interp as bass_interp
        sim = bass_interp.MultiCoreSim(nc, N_CORES)
        for d in range(N_CORES):
            for k, v in in_maps[d].items():
                sim.cores[d].tensor(k)[:] = np.asarray(v).reshape(
                    sim.cores[d].tensor(k).shape)
        sim.simulate(check_with_hw=False)
        outs = [np.asarray(sim.cores[d].mem_tensor("out")).reshape(s_hpad)[:s]
                for d in range(N_CORES)]
        return np.concatenate(outs).reshape(n, 1).astype(np.float32)
    global LAST_EXEC_NS
    import os
    n_iters = int(os.environ.get("BENCH_ITERS", "1"))
    results, best_ns = _run_spmd_timed(nc, in_maps, n_iters=n_iters)
    LAST_EXEC_NS = best_ns
    print(f"[kernel] exec done {time.time()-_t0:.0f}s", flush=True)
    outs = [results[d]["out"][:s, 0] for d in range(N_CORES)]
    return np.concatenate(outs).reshape(n, 1).astype(np.float32)
